# revision 25
# baseline (speedup 1.0000x reference)
"""Causal self-attention Bass/Tile kernel for 8 TRN2 NeuronCores.

Sharding: core c handles batch b = c//2 and heads h in [8*(c%2), 8*(c%2)+8).
Each core computes a partial projection output (its 512 channels' worth of the
contraction); the host sums the two partials per batch.

Design (software-pipelined, host-pretransposed):
  - Host pre-transposes x -> xT [C,T], wqkv -> wqkvT [C,3*JL], wproj ->
    wprojT [JL,C], all bf16: the kernel issues only plain contiguous DMAs.
  - All of xT + weights resident in SBUF; QKV for chunk q+1 and proj for
    chunk q-1 are emitted as PE "filler" groups interleaved into the
    attention loop of chunk q, so the PE never idles while ACT chews exp.
  - Scores: bf16 k/q, K=64 matmul pairs packed into row groups 0-1/2-3
    (base partitions 0/64), fp32 psum [128,2,512], no mask matmul. The
    attention loop processes j-tiles in PAIRS (two score pairs, then four
    AV matmuls) to amortize the K=64-pair -> K=128 array-drain transition;
    score/exp lookahead is 2 tiles (the 8-bank PSUM budget cap) and
    crosses head-pair boundaries.
  - Causal mask applied post-exp: GpSimd multiplies the 128-wide diagonal
    block of pt by a 0/1 lower-triangle bf16 tile (exp of the masked
    region is computed and then zeroed; scores there are valid finite
    values so no NaN/Inf risk).
  - AV accumulates [65,512] fp32 psum per head (ones column in v gives the
    softmax denominator l in row 64).
  - Softmax divide: l rows copied to SBUF, broadcast via K=1 f32r
    select-matmul into a psum bank, DVE reciprocal + multiply -> yT bf16
    for the proj matmuls. (Lessons from rejected variants: gpsimd
    partition_broadcast in this chain serializes the gpsimd queue with
    the AV-gating masks and costs ~200us; reciprocal_approx_fast read
    directly from a psum row at partition 64 returns garbage; merging the
    pair exps into one ACT coarsens pipeline granularity and costs 45us;
    interleaving chunk-2/3 attention rings races nondeterministically.)
  PSUM budget: scores 2x2 banks + AV 2x1 + mm 2x1 = 8 banks exactly.
"""

import sys
from collections import deque

if "/opt/trn_rl_repo" not in sys.path:
    sys.path.insert(0, "/opt/trn_rl_repo")

import ml_dtypes
import numpy as np

import concourse.bass as bass
import concourse.mybir as mybir
import concourse.tile as tile
from concourse import bacc, bass_utils

F32 = mybir.dt.float32
F32R = mybir.dt.float32r
BF16 = mybir.dt.bfloat16

B, T, C = 4, 2048, 1024
H = 16
D = 64
JL = 512          # local channels per q/k/v slice (8 heads * 64)
P = 128
NCHUNK = T // 512
CT = C // P       # 8 c-tiles


def build_nc():
    nc = bacc.Bacc("TRN2", target_bir_lowering=False, debug=False)
    xT_d = nc.dram_tensor("xT", [C, T], BF16, kind="ExternalInput").ap()
    wqkvT_d = nc.dram_tensor("wqkvT", [C, 3 * JL], BF16, kind="ExternalInput").ap()
    wprojT_d = nc.dram_tensor("wprojT", [JL, C], BF16, kind="ExternalInput").ap()
    out_d = nc.dram_tensor("out", [T, C], F32, kind="ExternalOutput").ap()

    Exp = mybir.ActivationFunctionType.Exp

    with tile.TileContext(nc) as tc:
        with (
            tc.tile_pool(name="singles", bufs=1) as singles,
            tc.tile_pool(name="qsb", bufs=2) as qsb,
            tc.tile_pool(name="ptp", bufs=4) as ptp,
            tc.tile_pool(name="ytp", bufs=2) as ytp,
            tc.tile_pool(name="obp", bufs=3) as obp,
            tc.tile_pool(name="smp", bufs=4) as smp,
            tc.tile_pool(name="ps_sc", bufs=2, space="PSUM") as ps_sc,
            tc.tile_pool(name="ps_av", bufs=2, space="PSUM") as ps_av,
            tc.tile_pool(name="ps_mm", bufs=2, space="PSUM") as ps_mm,
        ):
            # ---- persistent SBUF tensors ----
            xT_sb = singles.tile([P, CT, T], BF16)        # 32KB/part
            wq_sb = singles.tile([P, CT, 3 * JL], BF16)   # 24KB/part
            wp_sb = singles.tile([P, 4, C], BF16)         # 8KB/part
            k_sb = singles.tile([P, 4, T], BF16)          # 16KB/part
            v_sb = singles.tile([P, T // P, 8, D + 1], BF16)
            nc.vector.memset(v_sb[:, :, :, D], 1.0)

            # head-pair selector for the l broadcast: out rows 0:64 <- l0,
            # rows 64:128 <- l1  (out = sel.T @ [l0;l1])
            selaf = singles.tile([1, P], F32)
            nc.vector.memset(selaf, 0.0)
            nc.vector.memset(selaf[0:1, 0:D], 1.0)
            selbf = singles.tile([1, P], F32)
            nc.vector.memset(selbf, 0.0)
            nc.vector.memset(selbf[0:1, D:2 * D], 1.0)
            sel_a = singles.tile([1, P], F32R)
            nc.vector.tensor_copy(sel_a, selaf)
            sel_b = singles.tile([1, P], F32R)
            nc.vector.tensor_copy(sel_b, selbf)

            # 0/1 lower-triangle mask (keep f >= p), both h2 slots
            tri = singles.tile([P, 2, P], BF16)
            nc.gpsimd.memset(tri, 1.0)
            nc.gpsimd.affine_select(
                out=tri,
                in_=tri,
                pattern=[[0, 2], [1, P]],
                compare_op=mybir.AluOpType.is_ge,
                fill=0.0,
                base=0,
                channel_multiplier=-1,
            )

            # ---- input DMAs (plain, contiguous; q/k weights + chunk-0 x
            # first so the first QKV groups start ASAP) ----
            for cc in range(CT):
                nc.sync.dma_start(
                    out=wq_sb[:, cc, 0:2 * JL],
                    in_=wqkvT_d[cc * P:(cc + 1) * P, 0:2 * JL],
                )
                nc.sync.dma_start(
                    out=xT_sb[:, cc, 0:512], in_=xT_d[cc * P:(cc + 1) * P, 0:512]
                )
            for cc in range(CT):
                nc.sync.dma_start(
                    out=wq_sb[:, cc, 2 * JL:3 * JL],
                    in_=wqkvT_d[cc * P:(cc + 1) * P, 2 * JL:3 * JL],
                )
            for cc in range(CT):
                nc.sync.dma_start(
                    out=xT_sb[:, cc, 512:T], in_=xT_d[cc * P:(cc + 1) * P, 512:T]
                )
            for g in range(4):
                nc.sync.dma_start(
                    out=wp_sb[:, g, :], in_=wprojT_d[g * P:(g + 1) * P, :]
                )

            # ---- emission helpers ----
            q_tiles = {}
            yT_tiles = {}

            def emit_qk_group(q, hp, qt):
                t0 = q * 512
                for kind in range(2):            # 0 = q, 1 = k
                    col0 = kind * JL + hp * P
                    pq = ps_mm.tile([P, 512], F32, tag="mm", name="pq")
                    for cc in range(CT):
                        nc.tensor.matmul(
                            pq,
                            lhsT=wq_sb[:, cc, col0:col0 + P],
                            rhs=xT_sb[:, cc, t0:t0 + 512],
                            start=(cc == 0),
                            stop=(cc == CT - 1),
                        )
                    if kind == 0:
                        nc.vector.tensor_copy(out=qt[:, hp, :], in_=pq)
                    else:
                        nc.vector.tensor_copy(
                            out=k_sb[:, hp, t0:t0 + 512], in_=pq
                        )

            def emit_v_group(q, tt):
                t0 = q * 512
                pv = ps_mm.tile([P, 8, D], F32, tag="mm", name="pv")
                for cc in range(CT):
                    nc.tensor.matmul(
                        pv,
                        lhsT=xT_sb[:, cc, t0 + tt * P:t0 + (tt + 1) * P],
                        rhs=wq_sb[:, cc, 2 * JL:3 * JL],
                        start=(cc == 0),
                        stop=(cc == CT - 1),
                    )
                nc.vector.tensor_copy(
                    out=v_sb[:, q * 4 + tt, :, 0:D], in_=pv
                )

            def make_qkv_fillers(q):
                qt = qsb.tile([P, 4, 512], BF16, tag="q", name="qt")
                q_tiles[q] = qt
                fns = [
                    (lambda hp=hp: emit_qk_group(q, hp, qt)) for hp in range(4)
                ]
                fns += [(lambda tt=tt: emit_v_group(q, tt)) for tt in range(4)]
                return fns

            def emit_proj_group(q, tt, ct):
                t0 = q * 512
                yT_t = yT_tiles[q]
                po = ps_mm.tile([P, 512], F32, tag="mm", name="po")
                for g in range(4):
                    nc.tensor.matmul(
                        po,
                        lhsT=yT_t[:, g, tt * P:(tt + 1) * P],
                        rhs=wp_sb[:, g, ct * 512:(ct + 1) * 512],
                        start=(g == 0),
                        stop=(g == 3),
                    )
                obt = obp.tile([P, 512], F32, tag="ob", name="obt")
                nc.vector.tensor_copy(obt, po)
                nc.sync.dma_start(
                    out=out_d[
                        t0 + tt * P:t0 + (tt + 1) * P,
                        ct * 512:(ct + 1) * 512,
                    ],
                    in_=obt,
                )

            def make_proj_fillers(q):
                return [
                    (lambda tt=tt, ct=ct: emit_proj_group(q, tt, ct))
                    for tt in range(4) for ct in range(2)
                ]

            filler = deque()
            fstate = {"acc": 0.0, "rate": 0.0}

            def maybe_filler(force=0):
                fstate["acc"] += fstate["rate"]
                n = max(int(fstate["acc"]), force)
                while n > 0 and filler:
                    filler.popleft()()
                    fstate["acc"] = max(fstate["acc"] - 1.0, 0.0)
                    n -= 1

            def flush_fillers():
                while filler:
                    filler.popleft()()
                fstate["acc"] = 0.0

            def emit_attention_groups(groups, post_div=None):
                # groups: list of (chunk, hp); flat tile stream with
                # score/exp lookahead 2 crossing group boundaries.
                tiles = []
                for ch, hp in groups:
                    for j in range(4 * (ch + 1)):
                        tiles.append((ch, hp, j))
                nt = len(tiles)
                slots = nt // 2 + 8
                fstate["rate"] = len(filler) / slots if slots else 0.0
                fstate["acc"] = 0.0

                pav = {}   # (ch, hp) -> [pav0, pav1]
                cur = {}   # (ch, hp, j) -> (pt, o)

                def emit_sc_exp(ch, hp, j):
                    diag = j >= 4 * ch
                    o = j * P - ch * 512 if diag else 0
                    qt = q_tiles[ch]
                    ps = ps_sc.tile([P, 2, 512], F32, tag="sc", name="ps")
                    for h2 in range(2):
                        nc.tensor.matmul(
                            ps[:, h2, o:512],
                            lhsT=k_sb[
                                h2 * D:(h2 + 1) * D, hp, j * P:(j + 1) * P
                            ],
                            rhs=qt[h2 * D:(h2 + 1) * D, hp, o:512],
                            start=True,
                            stop=True,
                        )
                    pt = ptp.tile([P, 2, 512], BF16, tag="pt", name="pt")
                    nc.scalar.activation(
                        pt[:, :, o:512], ps[:, :, o:512], Exp, scale=0.125
                    )
                    if diag:
                        nc.gpsimd.tensor_mul(
                            pt[:, :, o:o + P], pt[:, :, o:o + P], tri
                        )
                    cur[(ch, hp, j)] = (pt, o)

                def emit_av(ch, hp, j):
                    ntk = 4 * (ch + 1)
                    if j == 0:
                        pav[(ch, hp)] = [
                            ps_av.tile([D + 1, 512], F32, tag="av", name="pav")
                            for _ in range(2)
                        ]
                    pt, o = cur.pop((ch, hp, j))
                    for h2 in range(2):
                        nc.tensor.matmul(
                            pav[(ch, hp)][h2][:, o:512],
                            lhsT=v_sb[:, j, hp * 2 + h2, :],
                            rhs=pt[:, h2, o:512],
                            start=(j == 0),
                            stop=(j == ntk - 1),
                        )

                def emit_div(ch, hp):
                    # l rows (psum row 64) -> sbuf, broadcast via K=1
                    # select-matmul into a mm psum bank, reciprocal, then
                    # yT = pav * (1/l). Keep the broadcast on PE + DVE:
                    # gpsimd is busy gating AV with the causal masks.
                    if ch not in yT_tiles:
                        yT_tiles[ch] = ytp.tile(
                            [P, 4, 512], BF16, tag="yT", name="yT_t"
                        )
                    yT_t = yT_tiles[ch]
                    pv = pav[(ch, hp)]
                    l2a = smp.tile([1, 512], F32R, tag="l2", name="l2a")
                    nc.vector.tensor_copy(l2a, pv[0][D:D + 1, :])
                    l2b = smp.tile([1, 512], F32R, tag="l2", name="l2b")
                    nc.vector.tensor_copy(l2b, pv[1][D:D + 1, :])
                    pb = ps_mm.tile([P, 512], F32, tag="mm", name="pb")
                    nc.tensor.matmul(pb, lhsT=sel_a, rhs=l2a,
                                     start=True, stop=False)
                    nc.tensor.matmul(pb, lhsT=sel_b, rhs=l2b,
                                     start=False, stop=True)
                    pbs = smp.tile([P, 512], F32, tag="pbs", name="pbs",
                                   bufs=2)
                    nc.vector.reciprocal_approx_fast(out=pbs, in_=pb)
                    nc.vector.tensor_mul(
                        yT_t[0:D, hp, :], pv[0][0:D, :], pbs[0:D, :]
                    )
                    nc.vector.tensor_mul(
                        yT_t[D:P, hp, :], pv[1][0:D, :], pbs[D:P, :]
                    )

                # prologue: scores/exp for tiles 0, 1
                emit_sc_exp(*tiles[0])
                emit_sc_exp(*tiles[1])
                for s in range(0, nt, 2):
                    if s + 2 < nt:
                        emit_sc_exp(*tiles[s + 2])
                    if s + 3 < nt:
                        emit_sc_exp(*tiles[s + 3])
                    ch, hp, j = tiles[s]
                    maybe_filler(force=2 if j == 0 else 0)
                    emit_av(*tiles[s])
                    emit_av(*tiles[s + 1])
                    if tiles[s + 1][2] == 4 * (ch + 1) - 1:
                        emit_div(ch, hp)
                        if post_div is not None:
                            post_div(ch, hp)

            # ---- main schedule ----
            f0 = make_qkv_fillers(0)
            for fn in f0[0:1] + f0[4:8]:   # qk hp0 + v groups chunk 0
                fn()
            filler.extend(f0[1:4])         # remaining qk groups chunk 0
            filler.extend(make_qkv_fillers(1))
            emit_attention_groups([(0, hp) for hp in range(4)])
            flush_fillers()

            for q in range(1, NCHUNK):
                # QKV fillers first: their q/k evacuations gate the next
                # stage's first score matmuls, proj gates nothing until the
                # stage after.
                if q + 1 < NCHUNK:
                    filler.extend(make_qkv_fillers(q + 1))
                filler.extend(make_proj_fillers(q - 1))
                emit_attention_groups([(q, hp) for hp in range(4)])
                flush_fillers()

            for fn in make_proj_fillers(NCHUNK - 1):
                fn()

    nc.compile()
    return nc


_NC = None


def _get_nc():
    global _NC
    if _NC is None:
        _NC = build_nc()
    return _NC


def _shard_inputs(x, w_attn, w_proj):
    bf16 = ml_dtypes.bfloat16
    xT_b = [
        np.ascontiguousarray(x[b].T).astype(bf16) for b in range(B)
    ]
    wq_s, wp_s = [], []
    for s in range(2):
        j0 = s * JL
        blocks = [w_attn[g * C + j0:g * C + j0 + JL, :] for g in range(3)]
        wq = np.concatenate(blocks, axis=0)            # [3*JL, C]
        wq_s.append(np.ascontiguousarray(wq.T).astype(bf16))   # [C, 3*JL]
        wp_s.append(
            np.ascontiguousarray(w_proj[:, j0:j0 + JL].T).astype(bf16)
        )                                              # [JL, C]
    return [
        {
            "xT": xT_b[c // 2],
            "wqkvT": wq_s[c % 2],
            "wprojT": wp_s[c % 2],
        }
        for c in range(8)
    ]


def run(x, w_attn, w_proj, **run_kwargs):
    """Run on 8 cores; returns (out [B,T,C], BassKernelResults)."""
    nc = _get_nc()
    in_maps = _shard_inputs(np.asarray(x), np.asarray(w_attn), np.asarray(w_proj))
    res = bass_utils.run_bass_kernel_spmd(
        nc, in_maps, core_ids=list(range(8)), **run_kwargs
    )
    out = np.empty((B, T, C), dtype=np.float32)
    for b in range(B):
        out[b] = res.results[2 * b]["out"] + res.results[2 * b + 1]["out"]
    return out, res


def kernel(x, w_attn, w_proj):
    return run(x, w_attn, w_proj)[0]
